# revision 33
# baseline (speedup 1.0000x reference)
"""Data-parallel Trainium2 kernel for the weighted classification loss.

loss = -mean_b sum_c w[b,c] * log(1 - softmax(reps @ W.T + b)[b,c])

Strategy (8 cores, batch-sharded 4096 rows each):
  - reps tiles stream HBM->SBUF with an in-flight f32->bf16 cast (SWDGE).
  - DVE StreamTranspose on int32-bitcast views puts D (in 32-chunks) on
    partitions; the K=32 matmuls consume that layout directly via APs,
    4-way row-group packed (tile_position), accumulating logits^T [10,N]
    in PSUM.
  - exp(l + bias) on ACT; one matmul with (ones - I | ones) computes
    u_c = den - e_c and den; ln on ACT; a matmul with the 10x10
    class-weight matrix (plus a -14*ln(den) row) yields Z[l, n]; a DVE
    scalar_tensor_tensor selects (labels == iota) * Z with a free-dim
    accumulate -> per-core partial sums; host combines.
  - The activation-table map is restricted so BOTH Exp and Ln resolve to
    the natural_log_exp_and_others set: exactly one ACT_TABLE_LOAD, no
    Exp<->Ln table swaps, so each slab's exp->u->ln->z->stt chain
    pipelines immediately.
  - Slab sizes [1024,1024,1024,512,512] with the last chunk split in two
    256-row DMAs keep the post-DMA tail short.
"""

import os
import sys

import numpy as np

if "/opt/trn_rl_repo" not in sys.path:
    sys.path.insert(0, "/opt/trn_rl_repo")

import ml_dtypes

B, D, C = 32768, 1024, 10
NCORES = 8
SHARD = B // NCORES  # 4096
# (base, rows): SWDGE cast-DMA chunks (rows 0-3584). Keep <=9 SWDGE
# dma_starts (a 10th overflows the descriptor ring and the tail
# transfer degrades badly).
CHUNK_DEFS = [
    (0, 512), (512, 512), (1024, 512), (1536, 512),
    (2048, 512), (2560, 512), (3072, 512),
]
# Rows 3584-4096 load as f32 on the (otherwise idle) HWDGE scalar-queue
# whose packets interleave 1:1 with the SWDGE stream, so they land
# mid-kernel instead of behind the whole SWDGE backlog; the Scalar
# engine casts them to bf16 in its idle window.
HW_CHUNKS = [(3584, 256), (3840, 256)]
# Slabs in EMISSION order: the hw-loaded slab computes second-to-last
# (its data is ready early), and the slab fed by the final SWDGE chunk
# (rows 3072-3584) is the tail with short 128-col downstream groups.
SLAB_DEFS = [
    (0, 512), (512, 1024), (1536, 1024), (2560, 512), (3584, 512), (3072, 512),
]
SLAB_GCOLS = [512, 512, 512, 512, 512, 128]
HW_SLAB = 4  # index into SLAB_DEFS of the hw-loaded slab
# global (group-index base, ncols) bookkeeping for the downstream stages
_G0 = []
SLAB_COL = []  # device-column offset of each slab (emission order)
_n = 0
_c = 0
for (_b, _r), _g in zip(SLAB_DEFS, SLAB_GCOLS):
    _G0.append(_n)
    SLAB_COL.append(_c)
    _n += _r // _g
    _c += _r
NGROUPS = _n  # 11
MID = 5
OPP_W = 2.0

_CACHE: dict = {}


def _pin_act_tables():
    """Restrict the activation-table map so Exp and Ln both resolve only
    to the natural_log_exp_and_others set (it contains both): the table
    insertion pass then emits a single ACT_TABLE_LOAD instead of
    ping-ponging Exp<->Ln sets. Set names/positions (= runtime ids) are
    preserved; only membership of the redundant sets is trimmed."""
    from concourse import bacc as bacc_mod
    from concourse import mybir

    real = bacc_mod.get_activation_tables

    def patched(arch):
        tables = {k: set(v) for k, v in real(arch).items()}
        exp_t = mybir.ActivationFunctionType.Exp
        ln_t = mybir.ActivationFunctionType.Ln
        assert exp_t in tables.get("natural_log_exp_and_others", set())
        assert ln_t in tables.get("natural_log_exp_and_others", set())
        for name, fns in tables.items():
            if name != "natural_log_exp_and_others":
                fns.discard(exp_t)
                fns.discard(ln_t)
        return tables

    bacc_mod.get_activation_tables = patched
    return bacc_mod, real


def _build_nc():
    from contextlib import ExitStack

    import concourse.mybir as mybir
    import concourse.tile as tile
    from concourse import bacc

    f32 = mybir.dt.float32
    bf16 = mybir.dt.bfloat16
    i32 = mybir.dt.int32
    Exp = mybir.ActivationFunctionType.Exp
    Ln = mybir.ActivationFunctionType.Ln
    Copy = mybir.ActivationFunctionType.Copy
    alu = mybir.AluOpType

    bacc_mod, real_tables = _pin_act_tables()
    try:
        nc = bacc.Bacc(
            "TRN2",
            target_bir_lowering=False,
            debug=False,
            enable_asserts=True,
            num_devices=NCORES,
        )
        reps = nc.dram_tensor("reps", [SHARD, D], f32, kind="ExternalInput").ap()
        labels_rep = nc.dram_tensor(
            "labels_rep", [C, SHARD], f32, kind="ExternalInput"
        ).ap()
        wta = nc.dram_tensor("wta", [128, 320], bf16, kind="ExternalInput").ap()
        uzw = nc.dram_tensor("uzw", [C, C + 1], bf16, kind="ExternalInput").ap()
        wz = nc.dram_tensor("wz", [C + 1, C], bf16, kind="ExternalInput").ap()
        iota = nc.dram_tensor("iota", [C, 1], f32, kind="ExternalInput").ap()
        biasc = nc.dram_tensor("biasc", [C, 1], f32, kind="ExternalInput").ap()
        partials = nc.dram_tensor(
            "partials", [C, NGROUPS], f32, kind="ExternalOutput"
        ).ap()

        with tile.TileContext(nc) as tc:
            with ExitStack() as ctx:
                const_pool = ctx.enter_context(tc.tile_pool(name="const", bufs=1))
                raw_pool = ctx.enter_context(tc.tile_pool(name="raw", bufs=6))
                rawf_pool = ctx.enter_context(tc.tile_pool(name="rawf", bufs=2))
                rawb_pool = ctx.enter_context(tc.tile_pool(name="rawb", bufs=2))
                scram_pool = ctx.enter_context(tc.tile_pool(name="scram", bufs=3))
                e_pool = ctx.enter_context(tc.tile_pool(name="e", bufs=2))
                ln_pool = ctx.enter_context(tc.tile_pool(name="lnu", bufs=4))
                scr_pool = ctx.enter_context(tc.tile_pool(name="scr", bufs=2))
                lp_pool = ctx.enter_context(
                    tc.tile_pool(name="lp", bufs=2, space="PSUM")
                )
                u_pool = ctx.enter_context(
                    tc.tile_pool(name="u", bufs=3, space="PSUM")
                )
                z_pool = ctx.enter_context(
                    tc.tile_pool(name="z", bufs=3, space="PSUM")
                )

                # consts on the HWDGE queue; wta first (gates first matmul),
                # labels last (only needed by the stt stage).
                wta_t = const_pool.tile([128, 320], bf16, tag="wta")
                nc.sync.dma_start(wta_t[:], wta)
                uzw_t = const_pool.tile([C, C + 1], bf16, tag="uzw")
                nc.sync.dma_start(uzw_t[:], uzw)
                wz_t = const_pool.tile([C + 1, C], bf16, tag="wz")
                nc.sync.dma_start(wz_t[:], wz)
                iota_t = const_pool.tile([C, 1], f32, tag="iota")
                nc.sync.dma_start(iota_t[:], iota)
                bias_t = const_pool.tile([C, 1], f32, tag="bias")
                nc.sync.dma_start(bias_t[:], biasc)
                lab_t = const_pool.tile([C, SHARD], f32, tag="lab")
                nc.sync.dma_start(lab_t[:], labels_rep)
                acc = const_pool.tile([C, NGROUPS], f32, tag="acc")

                # --- stream reps in, then DVE block-transpose each chunk
                # into its slab's scram tile.
                scram_tiles = {}
                for s, (sbase, srows) in enumerate(SLAB_DEFS):
                    scram_tiles[s] = scram_pool.tile(
                        [128, (srows // 128) * 512],
                        i32,
                        tag="scram",
                        name=f"scram{s}",
                    )

                def owning_slab(cbase):
                    return next(
                        i
                        for i, (sb, sr) in enumerate(SLAB_DEFS)
                        if sb <= cbase < sb + sr
                    )

                def emit_transposes(raw32, s, goff, tiles):
                    scram = scram_tiles[s]
                    # [128,1024]-i32 calls (2 row-tiles each)
                    for h in range(0, tiles, 2):
                        span = min(2, tiles - h) * 512
                        nc.vector.transpose(
                            scram[:, (goff + h) * 512 : (goff + h) * 512 + span],
                            raw32[:, h * 512 : h * 512 + span],
                        )

                # hw-queue f32 loads for the hw slab (issued early; they
                # interleave with the SWDGE stream instead of queueing
                # behind it)
                hw_raws = []
                for cbase, crows in HW_CHUNKS:
                    tiles = crows // 128
                    rawf = rawf_pool.tile(
                        [128, tiles * D], f32, tag="rawf", name=f"rawf{cbase}"
                    )
                    src = reps[cbase : cbase + crows, :].rearrange(
                        "(t p) d -> p t d", p=128
                    )
                    nc.scalar.dma_start(rawf[:], src)
                    hw_raws.append((rawf, cbase, tiles))

                deferred_T = []  # last SWDGE chunk's transposes go after
                # the hw slab's, right where the DVE would idle-wait anyway
                for ci, (cbase, crows) in enumerate(CHUNK_DEFS):
                    tiles = crows // 128
                    raw = raw_pool.tile([128, tiles * D], bf16, tag="raw")
                    src = reps[cbase : cbase + crows, :].rearrange(
                        "(t p) d -> p t d", p=128
                    )
                    nc.gpsimd.dma_start(raw[:], src)  # casts f32 -> bf16
                    raw32 = raw[:].bitcast(i32)
                    s = owning_slab(cbase)
                    goff = (cbase - SLAB_DEFS[s][0]) // 128
                    if ci == len(CHUNK_DEFS) - 1:
                        deferred_T.append((raw32, s, goff, tiles))
                    else:
                        emit_transposes(raw32, s, goff, tiles)

                def emit_hw_cast_and_transposes():
                    for rawf, cbase, tiles in hw_raws:
                        rawb = rawb_pool.tile(
                            [128, tiles * D],
                            bf16,
                            tag="rawb",
                            name=f"rawb{cbase}",
                        )
                        nc.scalar.activation(rawb[:], rawf[:], Copy)
                        s = owning_slab(cbase)
                        goff = (cbase - SLAB_DEFS[s][0]) // 128
                        emit_transposes(rawb[:].bitcast(i32), s, goff, tiles)
                    for args in deferred_T:
                        emit_transposes(*args)

                # --- per slab: packed matmuls -> exp; the u-matmuls of slab
                # s-1 and z-matmuls of slab s-2 are emitted after slab s's
                # logits matmuls so they never stall the in-order Tensor
                # queue on an Activation-engine dependency.
                def groups(s):
                    _, srows = SLAB_DEFS[s]
                    w = SLAB_GCOLS[s]
                    return [
                        (_G0[s] + gk, gk * w, w) for gk in range(srows // w)
                    ]

                e_tiles = {}
                u_tiles = {}
                ln_tiles = {}
                z_tiles = {}

                def emit_u(s):
                    e = e_tiles[s]
                    for k, col, w in groups(s):
                        sl = slice(col, col + w)
                        u = u_pool.tile([C + 1, w], f32, tag="u", name=f"u{k}")
                        nc.tensor.matmul(
                            u[:], uzw_t[:], e[:, sl], start=True, stop=True
                        )
                        u_tiles[k] = u

                def emit_ln(s):
                    for k, _, w in groups(s):
                        lnu = ln_pool.tile(
                            [C + 1, w], bf16, tag="lnu", name=f"ln{k}"
                        )
                        nc.scalar.activation(lnu[:], u_tiles[k][:], Ln)
                        ln_tiles[k] = lnu

                def emit_z_stt(s):
                    for k, col, w in groups(s):
                        z = z_pool.tile([C, w], f32, tag="z", name=f"z{k}")
                        nc.tensor.matmul(
                            z[:], wz_t[:], ln_tiles[k][:], start=True, stop=True
                        )
                        scr = scr_pool.tile([C, w], f32, tag="scr", name=f"scr{k}")
                        gcol = SLAB_COL[s] + col
                        nc.vector.scalar_tensor_tensor(
                            out=scr[:],
                            in0=lab_t[:, gcol : gcol + w],
                            scalar=iota_t[:],
                            in1=z[:],
                            op0=alu.is_equal,
                            op1=alu.mult,
                            accum_out=acc[:, k : k + 1],
                        )

                for s, (sbase, srows) in enumerate(SLAB_DEFS):
                    G = srows // 128
                    nb = srows // 4  # columns per P chain
                    scram = scram_tiles[s]
                    # scram bf16 view layout:
                    #   scram_bf[32P + r, 1024 g + 64 f2 + 2 c + q]
                    #     = bf16(reps[sbase + 128 g + 32 P + c, 64 f2 + 2 r + q])
                    sv = scram[:].bitcast(bf16)  # [128, G*1024]
                    view = sv.rearrange(
                        "k (g f2 c q) -> k g f2 c q", g=G, f2=16, c=32, q=2
                    )
                    # All 4 P chains share ONE PSUM bank at partition offsets
                    # 32P; diagonal tile_position keeps them concurrent.
                    lp = lp_pool.tile([128, nb], f32, tag="lp", name=f"lp{s}")
                    for f2 in range(16):
                        for q in range(2):
                            first = f2 == 0 and q == 0
                            last = f2 == 15 and q == 1
                            for P in range(4):
                                rhs = view[32 * P : 32 * P + 32, :, f2, :, q]
                                wcol = (2 * f2 + q) * 10
                                lhsT = wta_t[
                                    32 * P : 32 * P + 32, wcol : wcol + 10
                                ]
                                nc.tensor.matmul(
                                    lp[32 * P : 32 * P + C, :],
                                    lhsT,
                                    rhs,
                                    start=first,
                                    stop=last,
                                    tile_position=(32 * P, 32 * P),
                                    skip_group_check=True,
                                )

                    # e = exp(logits + bias_c); column n = P*nb + g*32 + c
                    e = e_pool.tile([C, srows], bf16, tag="e", name=f"e{s}")
                    for P in range(4):
                        nc.scalar.activation(
                            e[:, P * nb : (P + 1) * nb],
                            lp[32 * P : 32 * P + C, :],
                            Exp,
                            bias=bias_t[:],
                            scale=1.0,
                        )
                    e_tiles[s] = e

                    if s == 1:
                        # hw-slab casts + transposes land here: ACT does the
                        # casts right after exp1; DVE slots the hw (and then
                        # the final SWDGE) transposes after chunk 5's.
                        emit_hw_cast_and_transposes()
                    if s >= 1:
                        emit_u(s - 1)
                        emit_ln(s - 1)
                    if s >= 2:
                        emit_z_stt(s - 2)

                nslab = len(SLAB_DEFS)
                emit_u(nslab - 1)
                emit_ln(nslab - 1)
                emit_z_stt(nslab - 2)
                # ship all partials except the last slab's groups while the
                # final z/stt chain still runs
                k_last = _G0[-1]
                nc.sync.dma_start(partials[:, :k_last], acc[:, :k_last])
                emit_z_stt(nslab - 1)
                nc.sync.dma_start(partials[:, k_last:], acc[:, k_last:])

        nc.compile()
    finally:
        bacc_mod.get_activation_tables = real_tables
    return nc


def _prepare_static(W: np.ndarray, b: np.ndarray):
    # wta[32P + r, (2 f2 + q)*10 + cls] = bf16(W[cls, 64 f2 + 2 r + q])
    wta = np.zeros((128, 320), dtype=np.float32)
    for P in range(4):
        for r in range(32):
            for f2 in range(16):
                for q in range(2):
                    d = 64 * f2 + 2 * r + q
                    wta[32 * P + r, (2 * f2 + q) * 10 : (2 * f2 + q) * 10 + 10] = (
                        W[:, d]
                    )
    wta = wta.astype(ml_dtypes.bfloat16)

    # u = uzw.T @ e : rows 0..9 -> den - e_c, row 10 -> den
    uzw = np.ones((C, C + 1), dtype=np.float32)
    uzw[:, :C] -= np.eye(C, dtype=np.float32)
    uzw = uzw.astype(ml_dtypes.bfloat16)  # exact 0/1

    # wmat[c, l]: 0 if c==l, 2 if opposite half, else 1 ; extra row -14
    cc = np.arange(C)[:, None]
    ll = np.arange(C)[None, :]
    opp = (cc < MID) != (ll < MID)
    wmat = np.where(cc == ll, 0.0, np.where(opp, OPP_W, 1.0)).astype(np.float32)
    wz = np.concatenate(
        [wmat, np.full((1, C), -float(C + MID - 1), dtype=np.float32)], axis=0
    ).astype(ml_dtypes.bfloat16)  # exact small ints

    iota = np.arange(C, dtype=np.float32).reshape(C, 1)
    biasc = b.astype(np.float32).reshape(C, 1)
    return wta, uzw, wz, iota, biasc


def kernel(reps, W, b, labels):
    from concourse.bass_utils import run_bass_kernel_spmd

    reps = np.asarray(reps, dtype=np.float32)
    W = np.asarray(W, dtype=np.float32)
    b = np.asarray(b, dtype=np.float32)
    labels_np = np.asarray(labels)

    if "nc" not in _CACHE:
        _CACHE["nc"] = _build_nc()
    nc = _CACHE["nc"]

    wta, uzw, wz, iota, biasc = _prepare_static(W, b)

    in_maps = []
    for core in range(NCORES):
        sh = slice(core * SHARD, (core + 1) * SHARD)
        lab = labels_np[sh].astype(np.float32)
        # device column order within a slab is (P, g, c) for batch row
        # (g*128 + P*32 + c); permute labels to match, per slab.
        pieces = []
        for base, rows in SLAB_DEFS:
            g = rows // 128
            pieces.append(
                lab[base : base + rows]
                .reshape(g, 4, 32)
                .transpose(1, 0, 2)
                .reshape(rows)
            )
        lab_perm = np.concatenate(pieces)
        lab_rep = np.broadcast_to(lab_perm, (C, SHARD)).copy()
        in_maps.append(
            {
                "reps": np.ascontiguousarray(reps[sh]),
                "labels_rep": lab_rep,
                "wta": wta,
                "uzw": uzw,
                "wz": wz,
                "iota": iota,
                "biasc": biasc,
            }
        )

    trace = bool(int(os.environ.get("CC_KERNEL_TRACE", "0")))
    res = run_bass_kernel_spmd(
        nc, in_maps, core_ids=list(range(NCORES)), trace=trace
    )
    if trace:
        _CACHE["last_results"] = res

    total = np.float64(0.0)
    for core in range(NCORES):
        total += np.float64(res.results[core]["partials"].sum(dtype=np.float64))
    loss = -(total / B)
    return np.float32(loss)


# revision 37
# speedup vs baseline: 1.0391x; 1.0391x over previous
"""Data-parallel Trainium2 kernel for the weighted classification loss.

loss = -mean_b sum_c w[b,c] * log(1 - softmax(reps @ W.T + b)[b,c])

Strategy (8 cores, batch-sharded 4096 rows each):
  - reps tiles stream HBM->SBUF with an in-flight f32->bf16 cast (SWDGE).
  - DVE StreamTranspose on int32-bitcast views puts D (in 32-chunks) on
    partitions; the K=32 matmuls consume that layout directly via APs,
    4-way row-group packed (tile_position), accumulating logits^T [10,N]
    in PSUM.
  - exp(l + bias) on ACT; one matmul with (ones - I | ones) computes
    u_c = den - e_c and den; ln on ACT; a matmul with the 10x10
    class-weight matrix (plus a -14*ln(den) row) yields Z[l, n]; a DVE
    scalar_tensor_tensor selects (labels == iota) * Z with a free-dim
    accumulate -> per-core partial sums; host combines.
  - The activation-table map is restricted so BOTH Exp and Ln resolve to
    the natural_log_exp_and_others set: exactly one ACT_TABLE_LOAD, no
    Exp<->Ln table swaps, so each slab's exp->u->ln->z->stt chain
    pipelines immediately.
  - Slab sizes [1024,1024,1024,512,512] with the last chunk split in two
    256-row DMAs keep the post-DMA tail short.
"""

import os
import sys

import numpy as np

if "/opt/trn_rl_repo" not in sys.path:
    sys.path.insert(0, "/opt/trn_rl_repo")

import ml_dtypes

B, D, C = 32768, 1024, 10
NCORES = 8
SHARD = B // NCORES  # 4096
# (base, rows): SWDGE cast-DMA chunks (rows 0-3584). Keep <=9 SWDGE
# dma_starts (a 10th overflows the descriptor ring and the tail
# transfer degrades badly).
CHUNK_DEFS = [
    (0, 512), (512, 512), (1024, 512), (1536, 512),
    (2048, 512), (2560, 512), (3072, 512),
]
# Rows 3584-4096 load as f32 on the (otherwise idle) HWDGE scalar-queue
# whose packets interleave 1:1 with the SWDGE stream, so they land
# mid-kernel instead of behind the whole SWDGE backlog; the Scalar
# engine casts them to bf16 in its idle window.
HW_CHUNKS = [(3584, 256), (3840, 256)]
# Slabs in EMISSION order: the hw-loaded rows compute as slab 1 (their
# data cut the DMA line and is ready early, feeding the PE while the
# SWDGE stream ramps); the slab fed by the final SWDGE chunk (rows
# 3072-3584) is the tail with short 128-col downstream groups.
SLAB_DEFS = [
    (0, 512), (3584, 512), (512, 1024), (1536, 1024), (2560, 512), (3072, 512),
]
SLAB_GCOLS = [512, 512, 512, 512, 512, 128]
HW_SLAB = 1  # index into SLAB_DEFS of the hw-loaded slab
# global (group-index base, ncols) bookkeeping for the downstream stages
_G0 = []
SLAB_COL = []  # device-column offset of each slab (emission order)
_n = 0
_c = 0
for (_b, _r), _g in zip(SLAB_DEFS, SLAB_GCOLS):
    _G0.append(_n)
    SLAB_COL.append(_c)
    _n += _r // _g
    _c += _r
NGROUPS = _n  # 11
MID = 5
OPP_W = 2.0

_CACHE: dict = {}


def _pin_act_tables():
    """Restrict the activation-table map so Exp and Ln both resolve only
    to the natural_log_exp_and_others set (it contains both): the table
    insertion pass then emits a single ACT_TABLE_LOAD instead of
    ping-ponging Exp<->Ln sets. Set names/positions (= runtime ids) are
    preserved; only membership of the redundant sets is trimmed."""
    from concourse import bacc as bacc_mod
    from concourse import mybir

    real = bacc_mod.get_activation_tables

    def patched(arch):
        tables = {k: set(v) for k, v in real(arch).items()}
        pin = {
            mybir.ActivationFunctionType.Exp,
            mybir.ActivationFunctionType.Ln,
            mybir.ActivationFunctionType.Copy,
        }
        assert pin <= tables.get("natural_log_exp_and_others", set())
        for name, fns in tables.items():
            if name != "natural_log_exp_and_others":
                fns -= pin
        return tables

    bacc_mod.get_activation_tables = patched
    return bacc_mod, real


def _build_nc():
    from contextlib import ExitStack

    import concourse.mybir as mybir
    import concourse.tile as tile
    from concourse import bacc

    f32 = mybir.dt.float32
    bf16 = mybir.dt.bfloat16
    i32 = mybir.dt.int32
    Exp = mybir.ActivationFunctionType.Exp
    Ln = mybir.ActivationFunctionType.Ln
    Copy = mybir.ActivationFunctionType.Copy
    alu = mybir.AluOpType

    bacc_mod, real_tables = _pin_act_tables()
    try:
        nc = bacc.Bacc(
            "TRN2",
            target_bir_lowering=False,
            debug=False,
            enable_asserts=True,
            num_devices=NCORES,
        )
        reps = nc.dram_tensor("reps", [SHARD, D], f32, kind="ExternalInput").ap()
        labels_rep = nc.dram_tensor(
            "labels_rep", [C, SHARD], f32, kind="ExternalInput"
        ).ap()
        wta = nc.dram_tensor("wta", [128, 320], bf16, kind="ExternalInput").ap()
        uzw = nc.dram_tensor("uzw", [C, C + 1], bf16, kind="ExternalInput").ap()
        wz = nc.dram_tensor("wz", [C + 1, C], bf16, kind="ExternalInput").ap()
        iota = nc.dram_tensor("iota", [C, 1], f32, kind="ExternalInput").ap()
        biasc = nc.dram_tensor("biasc", [C, 1], f32, kind="ExternalInput").ap()
        partials = nc.dram_tensor(
            "partials", [C, NGROUPS], f32, kind="ExternalOutput"
        ).ap()

        with tile.TileContext(nc) as tc:
            with ExitStack() as ctx:
                const_pool = ctx.enter_context(tc.tile_pool(name="const", bufs=1))
                raw_pool = ctx.enter_context(tc.tile_pool(name="raw", bufs=6))
                rawf_pool = ctx.enter_context(tc.tile_pool(name="rawf", bufs=2))
                rawb_pool = ctx.enter_context(tc.tile_pool(name="rawb", bufs=2))
                scram_pool = ctx.enter_context(tc.tile_pool(name="scram", bufs=3))
                e_pool = ctx.enter_context(tc.tile_pool(name="e", bufs=2))
                ln_pool = ctx.enter_context(tc.tile_pool(name="lnu", bufs=4))
                scr_pool = ctx.enter_context(tc.tile_pool(name="scr", bufs=2))
                lp_pool = ctx.enter_context(
                    tc.tile_pool(name="lp", bufs=2, space="PSUM")
                )
                u_pool = ctx.enter_context(
                    tc.tile_pool(name="u", bufs=3, space="PSUM")
                )
                z_pool = ctx.enter_context(
                    tc.tile_pool(name="z", bufs=3, space="PSUM")
                )

                # consts on the HWDGE queue; wta first (gates first matmul),
                # labels last (only needed by the stt stage).
                wta_t = const_pool.tile([128, 320], bf16, tag="wta")
                nc.sync.dma_start(wta_t[:], wta)
                uzw_t = const_pool.tile([C, C + 1], bf16, tag="uzw")
                nc.sync.dma_start(uzw_t[:], uzw)
                wz_t = const_pool.tile([C + 1, C], bf16, tag="wz")
                nc.sync.dma_start(wz_t[:], wz)
                iota_t = const_pool.tile([C, 1], f32, tag="iota")
                nc.sync.dma_start(iota_t[:], iota)
                bias_t = const_pool.tile([C, 1], f32, tag="bias")
                nc.sync.dma_start(bias_t[:], biasc)
                lab_t = const_pool.tile([C, SHARD], f32, tag="lab")
                nc.sync.dma_start(lab_t[:], labels_rep)
                acc = const_pool.tile([C, NGROUPS], f32, tag="acc")

                # --- stream reps in, then DVE block-transpose each chunk
                # into its slab's scram tile.
                scram_tiles = {}
                for s, (sbase, srows) in enumerate(SLAB_DEFS):
                    scram_tiles[s] = scram_pool.tile(
                        [128, (srows // 128) * 512],
                        i32,
                        tag="scram",
                        name=f"scram{s}",
                    )

                def owning_slab(cbase):
                    return next(
                        i
                        for i, (sb, sr) in enumerate(SLAB_DEFS)
                        if sb <= cbase < sb + sr
                    )

                def emit_transposes(raw32, s, goff, tiles):
                    scram = scram_tiles[s]
                    # [128,1024]-i32 calls (2 row-tiles each)
                    for h in range(0, tiles, 2):
                        span = min(2, tiles - h) * 512
                        nc.vector.transpose(
                            scram[:, (goff + h) * 512 : (goff + h) * 512 + span],
                            raw32[:, h * 512 : h * 512 + span],
                        )

                # hw-queue f32 loads for the hw slab (issued early; they
                # interleave with the SWDGE stream instead of queueing
                # behind it)
                hw_raws = []
                for cbase, crows in HW_CHUNKS:
                    tiles = crows // 128
                    rawf = rawf_pool.tile(
                        [128, tiles * D], f32, tag="rawf", name=f"rawf{cbase}"
                    )
                    src = reps[cbase : cbase + crows, :].rearrange(
                        "(t p) d -> p t d", p=128
                    )
                    nc.scalar.dma_start(rawf[:], src)
                    hw_raws.append((rawf, cbase, tiles))

                # all SWDGE chunk loads up front; transposes are emitted in
                # the DVE order [T(ch0), Thw..., T(ch1..ch6)] below
                chunk_T = []
                for cbase, crows in CHUNK_DEFS:
                    tiles = crows // 128
                    raw = raw_pool.tile([128, tiles * D], bf16, tag="raw")
                    src = reps[cbase : cbase + crows, :].rearrange(
                        "(t p) d -> p t d", p=128
                    )
                    nc.gpsimd.dma_start(raw[:], src)  # casts f32 -> bf16
                    s = owning_slab(cbase)
                    goff = (cbase - SLAB_DEFS[s][0]) // 128
                    chunk_T.append((raw[:].bitcast(i32), s, goff, tiles))

                emit_transposes(*chunk_T[0])
                # hw rows: Scalar-engine cast f32->bf16 (Copy shares the
                # pinned table set), then the same pair-transpose
                for rawf, cbase, tiles in hw_raws:
                    rawb = rawb_pool.tile(
                        [128, tiles * D], bf16, tag="rawb", name=f"rawb{cbase}"
                    )
                    nc.scalar.activation(rawb[:], rawf[:], Copy)
                    s = owning_slab(cbase)
                    goff = (cbase - SLAB_DEFS[s][0]) // 128
                    emit_transposes(rawb[:].bitcast(i32), s, goff, tiles)
                for args in chunk_T[1:]:
                    emit_transposes(*args)

                # --- per slab: packed matmuls -> exp; the u-matmuls of slab
                # s-1 and z-matmuls of slab s-2 are emitted after slab s's
                # logits matmuls so they never stall the in-order Tensor
                # queue on an Activation-engine dependency.
                def groups(s):
                    _, srows = SLAB_DEFS[s]
                    w = SLAB_GCOLS[s]
                    return [
                        (_G0[s] + gk, gk * w, w) for gk in range(srows // w)
                    ]

                e_tiles = {}
                u_tiles = {}
                ln_tiles = {}
                z_tiles = {}

                def emit_u(s):
                    e = e_tiles[s]
                    for k, col, w in groups(s):
                        sl = slice(col, col + w)
                        u = u_pool.tile([C + 1, w], f32, tag="u", name=f"u{k}")
                        nc.tensor.matmul(
                            u[:], uzw_t[:], e[:, sl], start=True, stop=True
                        )
                        u_tiles[k] = u

                def emit_ln(s):
                    for k, _, w in groups(s):
                        lnu = ln_pool.tile(
                            [C + 1, w], bf16, tag="lnu", name=f"ln{k}"
                        )
                        nc.scalar.activation(lnu[:], u_tiles[k][:], Ln)
                        ln_tiles[k] = lnu

                def emit_z_stt(s):
                    for k, col, w in groups(s):
                        z = z_pool.tile([C, w], f32, tag="z", name=f"z{k}")
                        nc.tensor.matmul(
                            z[:], wz_t[:], ln_tiles[k][:], start=True, stop=True
                        )
                        scr = scr_pool.tile([C, w], f32, tag="scr", name=f"scr{k}")
                        gcol = SLAB_COL[s] + col
                        nc.vector.scalar_tensor_tensor(
                            out=scr[:],
                            in0=lab_t[:, gcol : gcol + w],
                            scalar=iota_t[:],
                            in1=z[:],
                            op0=alu.is_equal,
                            op1=alu.mult,
                            accum_out=acc[:, k : k + 1],
                        )

                for s, (sbase, srows) in enumerate(SLAB_DEFS):
                    G = srows // 128
                    nb = srows // 4  # columns per P chain
                    scram = scram_tiles[s]
                    # scram bf16 view layout:
                    #   scram_bf[32P + r, 1024 g + 64 f2 + 2 c + q]
                    #     = bf16(reps[sbase + 128 g + 32 P + c, 64 f2 + 2 r + q])
                    sv = scram[:].bitcast(bf16)  # [128, G*1024]
                    view = sv.rearrange(
                        "k (g f2 c q) -> k g f2 c q", g=G, f2=16, c=32, q=2
                    )
                    # All 4 P chains share ONE PSUM bank at partition offsets
                    # 32P; diagonal tile_position keeps them concurrent.
                    lp = lp_pool.tile([128, nb], f32, tag="lp", name=f"lp{s}")
                    for f2 in range(16):
                        for q in range(2):
                            first = f2 == 0 and q == 0
                            last = f2 == 15 and q == 1
                            for P in range(4):
                                rhs = view[32 * P : 32 * P + 32, :, f2, :, q]
                                wcol = (2 * f2 + q) * 10
                                lhsT = wta_t[
                                    32 * P : 32 * P + 32, wcol : wcol + 10
                                ]
                                nc.tensor.matmul(
                                    lp[32 * P : 32 * P + C, :],
                                    lhsT,
                                    rhs,
                                    start=first,
                                    stop=last,
                                    tile_position=(32 * P, 32 * P),
                                    skip_group_check=True,
                                )

                    # e = exp(logits + bias_c); column n = P*nb + g*32 + c
                    e = e_pool.tile([C, srows], bf16, tag="e", name=f"e{s}")
                    for P in range(4):
                        nc.scalar.activation(
                            e[:, P * nb : (P + 1) * nb],
                            lp[32 * P : 32 * P + C, :],
                            Exp,
                            bias=bias_t[:],
                            scale=1.0,
                        )
                    e_tiles[s] = e

                    if s >= 1:
                        emit_u(s - 1)
                        emit_ln(s - 1)
                    if s >= 2:
                        emit_z_stt(s - 2)

                nslab = len(SLAB_DEFS)
                emit_u(nslab - 1)
                emit_ln(nslab - 1)
                emit_z_stt(nslab - 2)
                # ship all partials except the last slab's groups while the
                # final z/stt chain still runs
                k_last = _G0[-1]
                nc.sync.dma_start(partials[:, :k_last], acc[:, :k_last])
                emit_z_stt(nslab - 1)
                nc.sync.dma_start(partials[:, k_last:], acc[:, k_last:])

        nc.compile()
    finally:
        bacc_mod.get_activation_tables = real_tables
    return nc


def _prepare_static(W: np.ndarray, b: np.ndarray):
    # wta[32P + r, (2 f2 + q)*10 + cls] = bf16(W[cls, 64 f2 + 2 r + q])
    wta = np.zeros((128, 320), dtype=np.float32)
    for P in range(4):
        for r in range(32):
            for f2 in range(16):
                for q in range(2):
                    d = 64 * f2 + 2 * r + q
                    wta[32 * P + r, (2 * f2 + q) * 10 : (2 * f2 + q) * 10 + 10] = (
                        W[:, d]
                    )
    wta = wta.astype(ml_dtypes.bfloat16)

    # u = uzw.T @ e : rows 0..9 -> den - e_c, row 10 -> den
    uzw = np.ones((C, C + 1), dtype=np.float32)
    uzw[:, :C] -= np.eye(C, dtype=np.float32)
    uzw = uzw.astype(ml_dtypes.bfloat16)  # exact 0/1

    # wmat[c, l]: 0 if c==l, 2 if opposite half, else 1 ; extra row -14
    cc = np.arange(C)[:, None]
    ll = np.arange(C)[None, :]
    opp = (cc < MID) != (ll < MID)
    wmat = np.where(cc == ll, 0.0, np.where(opp, OPP_W, 1.0)).astype(np.float32)
    wz = np.concatenate(
        [wmat, np.full((1, C), -float(C + MID - 1), dtype=np.float32)], axis=0
    ).astype(ml_dtypes.bfloat16)  # exact small ints

    iota = np.arange(C, dtype=np.float32).reshape(C, 1)
    biasc = b.astype(np.float32).reshape(C, 1)
    return wta, uzw, wz, iota, biasc


def kernel(reps, W, b, labels):
    from concourse.bass_utils import run_bass_kernel_spmd

    reps = np.asarray(reps, dtype=np.float32)
    W = np.asarray(W, dtype=np.float32)
    b = np.asarray(b, dtype=np.float32)
    labels_np = np.asarray(labels)

    if "nc" not in _CACHE:
        _CACHE["nc"] = _build_nc()
    nc = _CACHE["nc"]

    wta, uzw, wz, iota, biasc = _prepare_static(W, b)

    in_maps = []
    for core in range(NCORES):
        sh = slice(core * SHARD, (core + 1) * SHARD)
        lab = labels_np[sh].astype(np.float32)
        # device column order within a slab is (P, g, c) for batch row
        # (g*128 + P*32 + c); permute labels to match, per slab.
        pieces = []
        for base, rows in SLAB_DEFS:
            g = rows // 128
            pieces.append(
                lab[base : base + rows]
                .reshape(g, 4, 32)
                .transpose(1, 0, 2)
                .reshape(rows)
            )
        lab_perm = np.concatenate(pieces)
        lab_rep = np.broadcast_to(lab_perm, (C, SHARD)).copy()
        in_maps.append(
            {
                "reps": np.ascontiguousarray(reps[sh]),
                "labels_rep": lab_rep,
                "wta": wta,
                "uzw": uzw,
                "wz": wz,
                "iota": iota,
                "biasc": biasc,
            }
        )

    trace = bool(int(os.environ.get("CC_KERNEL_TRACE", "0")))
    res = run_bass_kernel_spmd(
        nc, in_maps, core_ids=list(range(NCORES)), trace=trace
    )
    if trace:
        _CACHE["last_results"] = res

    total = np.float64(0.0)
    for core in range(NCORES):
        total += np.float64(res.results[core]["partials"].sum(dtype=np.float64))
    loss = -(total / B)
    return np.float32(loss)


# revision 38
# speedup vs baseline: 1.0523x; 1.0127x over previous
"""Data-parallel Trainium2 kernel for the weighted classification loss.

loss = -mean_b sum_c w[b,c] * log(1 - softmax(reps @ W.T + b)[b,c])

Strategy (8 cores, batch-sharded 4096 rows each):
  - reps tiles stream HBM->SBUF with an in-flight f32->bf16 cast (SWDGE).
  - DVE StreamTranspose on int32-bitcast views puts D (in 32-chunks) on
    partitions; the K=32 matmuls consume that layout directly via APs,
    4-way row-group packed (tile_position), accumulating logits^T [10,N]
    in PSUM.
  - exp(l + bias) on ACT; one matmul with (ones - I | ones) computes
    u_c = den - e_c and den; ln on ACT; a matmul with the 10x10
    class-weight matrix (plus a -14*ln(den) row) yields Z[l, n]; a DVE
    scalar_tensor_tensor selects (labels == iota) * Z with a free-dim
    accumulate -> per-core partial sums; host combines.
  - The activation-table map is restricted so BOTH Exp and Ln resolve to
    the natural_log_exp_and_others set: exactly one ACT_TABLE_LOAD, no
    Exp<->Ln table swaps, so each slab's exp->u->ln->z->stt chain
    pipelines immediately.
  - Slab sizes [1024,1024,1024,512,512] with the last chunk split in two
    256-row DMAs keep the post-DMA tail short.
"""

import os
import sys

import numpy as np

if "/opt/trn_rl_repo" not in sys.path:
    sys.path.insert(0, "/opt/trn_rl_repo")

import ml_dtypes

B, D, C = 32768, 1024, 10
NCORES = 8
SHARD = B // NCORES  # 4096
# (base, rows): SWDGE cast-DMA chunks (rows 0-3584). Keep <=9 SWDGE
# dma_starts (a 10th overflows the descriptor ring and the tail
# transfer degrades badly).
CHUNK_DEFS = [
    (0, 512), (512, 512), (1024, 512), (1536, 512),
    (2048, 512), (2560, 512), (3072, 256), (3328, 256),
]
# Rows 3584-4096 load as f32 on the (otherwise idle) HWDGE scalar-queue
# whose packets interleave 1:1 with the SWDGE stream, so they land
# mid-kernel instead of behind the whole SWDGE backlog; the Scalar
# engine casts them to bf16 in its idle window.
HW_CHUNKS = [(3584, 256), (3840, 256)]
# Slabs in EMISSION order: the hw-loaded rows compute as slab 1 (their
# data cut the DMA line and is ready early, feeding the PE while the
# SWDGE stream ramps); the slab fed by the final SWDGE chunk (rows
# 3072-3584) is the tail with short 128-col downstream groups.
SLAB_DEFS = [
    (0, 512), (3584, 512), (512, 1024), (1536, 1024), (2560, 512), (3072, 512),
]
SLAB_GCOLS = [512, 512, 512, 512, 512, 128]
HW_SLAB = 1  # index into SLAB_DEFS of the hw-loaded slab
# global (group-index base, ncols) bookkeeping for the downstream stages
_G0 = []
SLAB_COL = []  # device-column offset of each slab (emission order)
_n = 0
_c = 0
for (_b, _r), _g in zip(SLAB_DEFS, SLAB_GCOLS):
    _G0.append(_n)
    SLAB_COL.append(_c)
    _n += _r // _g
    _c += _r
NGROUPS = _n  # 11
MID = 5
OPP_W = 2.0

_CACHE: dict = {}


def _pin_act_tables():
    """Restrict the activation-table map so Exp and Ln both resolve only
    to the natural_log_exp_and_others set (it contains both): the table
    insertion pass then emits a single ACT_TABLE_LOAD instead of
    ping-ponging Exp<->Ln sets. Set names/positions (= runtime ids) are
    preserved; only membership of the redundant sets is trimmed."""
    from concourse import bacc as bacc_mod
    from concourse import mybir

    real = bacc_mod.get_activation_tables

    def patched(arch):
        tables = {k: set(v) for k, v in real(arch).items()}
        pin = {
            mybir.ActivationFunctionType.Exp,
            mybir.ActivationFunctionType.Ln,
            mybir.ActivationFunctionType.Copy,
        }
        assert pin <= tables.get("natural_log_exp_and_others", set())
        for name, fns in tables.items():
            if name != "natural_log_exp_and_others":
                fns -= pin
        return tables

    bacc_mod.get_activation_tables = patched
    return bacc_mod, real


def _build_nc():
    from contextlib import ExitStack

    import concourse.mybir as mybir
    import concourse.tile as tile
    from concourse import bacc

    f32 = mybir.dt.float32
    bf16 = mybir.dt.bfloat16
    i32 = mybir.dt.int32
    Exp = mybir.ActivationFunctionType.Exp
    Ln = mybir.ActivationFunctionType.Ln
    Copy = mybir.ActivationFunctionType.Copy
    alu = mybir.AluOpType

    bacc_mod, real_tables = _pin_act_tables()
    try:
        nc = bacc.Bacc(
            "TRN2",
            target_bir_lowering=False,
            debug=False,
            enable_asserts=True,
            num_devices=NCORES,
        )
        reps = nc.dram_tensor("reps", [SHARD, D], f32, kind="ExternalInput").ap()
        labels_rep = nc.dram_tensor(
            "labels_rep", [C, SHARD], f32, kind="ExternalInput"
        ).ap()
        wta = nc.dram_tensor("wta", [128, 320], bf16, kind="ExternalInput").ap()
        uzw = nc.dram_tensor("uzw", [C, C + 1], bf16, kind="ExternalInput").ap()
        wz = nc.dram_tensor("wz", [C + 1, C], bf16, kind="ExternalInput").ap()
        iota = nc.dram_tensor("iota", [C, 1], f32, kind="ExternalInput").ap()
        biasc = nc.dram_tensor("biasc", [C, 1], f32, kind="ExternalInput").ap()
        partials = nc.dram_tensor(
            "partials", [C, NGROUPS], f32, kind="ExternalOutput"
        ).ap()

        with tile.TileContext(nc) as tc:
            with ExitStack() as ctx:
                const_pool = ctx.enter_context(tc.tile_pool(name="const", bufs=1))
                raw_pool = ctx.enter_context(tc.tile_pool(name="raw", bufs=6))
                rawf_pool = ctx.enter_context(tc.tile_pool(name="rawf", bufs=2))
                rawb_pool = ctx.enter_context(tc.tile_pool(name="rawb", bufs=2))
                scram_pool = ctx.enter_context(tc.tile_pool(name="scram", bufs=3))
                e_pool = ctx.enter_context(tc.tile_pool(name="e", bufs=2))
                ln_pool = ctx.enter_context(tc.tile_pool(name="lnu", bufs=4))
                scr_pool = ctx.enter_context(tc.tile_pool(name="scr", bufs=2))
                lp_pool = ctx.enter_context(
                    tc.tile_pool(name="lp", bufs=2, space="PSUM")
                )
                u_pool = ctx.enter_context(
                    tc.tile_pool(name="u", bufs=3, space="PSUM")
                )
                z_pool = ctx.enter_context(
                    tc.tile_pool(name="z", bufs=3, space="PSUM")
                )

                # consts on the HWDGE queue; wta first (gates first matmul),
                # labels last (only needed by the stt stage).
                wta_t = const_pool.tile([128, 320], bf16, tag="wta")
                nc.sync.dma_start(wta_t[:], wta)
                uzw_t = const_pool.tile([C, C + 1], bf16, tag="uzw")
                nc.sync.dma_start(uzw_t[:], uzw)
                wz_t = const_pool.tile([C + 1, C], bf16, tag="wz")
                nc.sync.dma_start(wz_t[:], wz)
                iota_t = const_pool.tile([C, 1], f32, tag="iota")
                nc.sync.dma_start(iota_t[:], iota)
                bias_t = const_pool.tile([C, 1], f32, tag="bias")
                nc.sync.dma_start(bias_t[:], biasc)
                lab_t = const_pool.tile([C, SHARD], f32, tag="lab")
                nc.sync.dma_start(lab_t[:], labels_rep)
                acc = const_pool.tile([C, NGROUPS], f32, tag="acc")

                # --- stream reps in, then DVE block-transpose each chunk
                # into its slab's scram tile.
                scram_tiles = {}
                for s, (sbase, srows) in enumerate(SLAB_DEFS):
                    scram_tiles[s] = scram_pool.tile(
                        [128, (srows // 128) * 512],
                        i32,
                        tag="scram",
                        name=f"scram{s}",
                    )

                def owning_slab(cbase):
                    return next(
                        i
                        for i, (sb, sr) in enumerate(SLAB_DEFS)
                        if sb <= cbase < sb + sr
                    )

                def emit_transposes(raw32, s, goff, tiles):
                    scram = scram_tiles[s]
                    # [128,1024]-i32 calls (2 row-tiles each)
                    for h in range(0, tiles, 2):
                        span = min(2, tiles - h) * 512
                        nc.vector.transpose(
                            scram[:, (goff + h) * 512 : (goff + h) * 512 + span],
                            raw32[:, h * 512 : h * 512 + span],
                        )

                # hw-queue f32 loads for the hw slab (issued early; they
                # interleave with the SWDGE stream instead of queueing
                # behind it)
                hw_raws = []
                for cbase, crows in HW_CHUNKS:
                    tiles = crows // 128
                    rawf = rawf_pool.tile(
                        [128, tiles * D], f32, tag="rawf", name=f"rawf{cbase}"
                    )
                    src = reps[cbase : cbase + crows, :].rearrange(
                        "(t p) d -> p t d", p=128
                    )
                    nc.scalar.dma_start(rawf[:], src)
                    hw_raws.append((rawf, cbase, tiles))

                # all SWDGE chunk loads up front; transposes are emitted in
                # the DVE order [T(ch0), Thw..., T(ch1..ch6)] below
                chunk_T = []
                for cbase, crows in CHUNK_DEFS:
                    tiles = crows // 128
                    raw = raw_pool.tile([128, tiles * D], bf16, tag="raw")
                    src = reps[cbase : cbase + crows, :].rearrange(
                        "(t p) d -> p t d", p=128
                    )
                    nc.gpsimd.dma_start(raw[:], src)  # casts f32 -> bf16
                    s = owning_slab(cbase)
                    goff = (cbase - SLAB_DEFS[s][0]) // 128
                    chunk_T.append((raw[:].bitcast(i32), s, goff, tiles))

                emit_transposes(*chunk_T[0])
                # hw rows: Scalar-engine cast f32->bf16 (Copy shares the
                # pinned table set), then the same pair-transpose
                for rawf, cbase, tiles in hw_raws:
                    rawb = rawb_pool.tile(
                        [128, tiles * D], bf16, tag="rawb", name=f"rawb{cbase}"
                    )
                    nc.scalar.activation(rawb[:], rawf[:], Copy)
                    s = owning_slab(cbase)
                    goff = (cbase - SLAB_DEFS[s][0]) // 128
                    emit_transposes(rawb[:].bitcast(i32), s, goff, tiles)
                for args in chunk_T[1:]:
                    emit_transposes(*args)

                # --- per slab: packed matmuls -> exp; the u-matmuls of slab
                # s-1 and z-matmuls of slab s-2 are emitted after slab s's
                # logits matmuls so they never stall the in-order Tensor
                # queue on an Activation-engine dependency.
                def groups(s):
                    _, srows = SLAB_DEFS[s]
                    w = SLAB_GCOLS[s]
                    return [
                        (_G0[s] + gk, gk * w, w) for gk in range(srows // w)
                    ]

                e_tiles = {}
                u_tiles = {}
                ln_tiles = {}
                z_tiles = {}

                def emit_u(s):
                    e = e_tiles[s]
                    for k, col, w in groups(s):
                        sl = slice(col, col + w)
                        u = u_pool.tile([C + 1, w], f32, tag="u", name=f"u{k}")
                        nc.tensor.matmul(
                            u[:], uzw_t[:], e[:, sl], start=True, stop=True
                        )
                        u_tiles[k] = u

                def emit_ln(s):
                    for k, _, w in groups(s):
                        lnu = ln_pool.tile(
                            [C + 1, w], bf16, tag="lnu", name=f"ln{k}"
                        )
                        nc.scalar.activation(lnu[:], u_tiles[k][:], Ln)
                        ln_tiles[k] = lnu

                def emit_z_stt(s):
                    for k, col, w in groups(s):
                        z = z_pool.tile([C, w], f32, tag="z", name=f"z{k}")
                        nc.tensor.matmul(
                            z[:], wz_t[:], ln_tiles[k][:], start=True, stop=True
                        )
                        scr = scr_pool.tile([C, w], f32, tag="scr", name=f"scr{k}")
                        gcol = SLAB_COL[s] + col
                        nc.vector.scalar_tensor_tensor(
                            out=scr[:],
                            in0=lab_t[:, gcol : gcol + w],
                            scalar=iota_t[:],
                            in1=z[:],
                            op0=alu.is_equal,
                            op1=alu.mult,
                            accum_out=acc[:, k : k + 1],
                        )

                for s, (sbase, srows) in enumerate(SLAB_DEFS):
                    G = srows // 128
                    nb = srows // 4  # columns per P chain
                    scram = scram_tiles[s]
                    # scram bf16 view layout:
                    #   scram_bf[32P + r, 1024 g + 64 f2 + 2 c + q]
                    #     = bf16(reps[sbase + 128 g + 32 P + c, 64 f2 + 2 r + q])
                    sv = scram[:].bitcast(bf16)  # [128, G*1024]
                    view = sv.rearrange(
                        "k (g f2 c q) -> k g f2 c q", g=G, f2=16, c=32, q=2
                    )
                    # All 4 P chains share ONE PSUM bank at partition offsets
                    # 32P; diagonal tile_position keeps them concurrent.
                    lp = lp_pool.tile([128, nb], f32, tag="lp", name=f"lp{s}")
                    for f2 in range(16):
                        for q in range(2):
                            first = f2 == 0 and q == 0
                            last = f2 == 15 and q == 1
                            for P in range(4):
                                rhs = view[32 * P : 32 * P + 32, :, f2, :, q]
                                wcol = (2 * f2 + q) * 10
                                lhsT = wta_t[
                                    32 * P : 32 * P + 32, wcol : wcol + 10
                                ]
                                nc.tensor.matmul(
                                    lp[32 * P : 32 * P + C, :],
                                    lhsT,
                                    rhs,
                                    start=first,
                                    stop=last,
                                    tile_position=(32 * P, 32 * P),
                                    skip_group_check=True,
                                )

                    # e = exp(logits + bias_c); column n = P*nb + g*32 + c
                    e = e_pool.tile([C, srows], bf16, tag="e", name=f"e{s}")
                    for P in range(4):
                        nc.scalar.activation(
                            e[:, P * nb : (P + 1) * nb],
                            lp[32 * P : 32 * P + C, :],
                            Exp,
                            bias=bias_t[:],
                            scale=1.0,
                        )
                    e_tiles[s] = e

                    if s >= 1:
                        emit_u(s - 1)
                        emit_ln(s - 1)
                    if s >= 2:
                        emit_z_stt(s - 2)

                nslab = len(SLAB_DEFS)
                emit_u(nslab - 1)
                emit_ln(nslab - 1)
                emit_z_stt(nslab - 2)
                # ship all partials except the last slab's groups while the
                # final z/stt chain still runs
                k_last = _G0[-1]
                nc.sync.dma_start(partials[:, :k_last], acc[:, :k_last])
                emit_z_stt(nslab - 1)
                nc.sync.dma_start(partials[:, k_last:], acc[:, k_last:])

        nc.compile()
    finally:
        bacc_mod.get_activation_tables = real_tables
    return nc


def _prepare_static(W: np.ndarray, b: np.ndarray):
    # wta[32P + r, (2 f2 + q)*10 + cls] = bf16(W[cls, 64 f2 + 2 r + q])
    wta = np.zeros((128, 320), dtype=np.float32)
    for P in range(4):
        for r in range(32):
            for f2 in range(16):
                for q in range(2):
                    d = 64 * f2 + 2 * r + q
                    wta[32 * P + r, (2 * f2 + q) * 10 : (2 * f2 + q) * 10 + 10] = (
                        W[:, d]
                    )
    wta = wta.astype(ml_dtypes.bfloat16)

    # u = uzw.T @ e : rows 0..9 -> den - e_c, row 10 -> den
    uzw = np.ones((C, C + 1), dtype=np.float32)
    uzw[:, :C] -= np.eye(C, dtype=np.float32)
    uzw = uzw.astype(ml_dtypes.bfloat16)  # exact 0/1

    # wmat[c, l]: 0 if c==l, 2 if opposite half, else 1 ; extra row -14
    cc = np.arange(C)[:, None]
    ll = np.arange(C)[None, :]
    opp = (cc < MID) != (ll < MID)
    wmat = np.where(cc == ll, 0.0, np.where(opp, OPP_W, 1.0)).astype(np.float32)
    wz = np.concatenate(
        [wmat, np.full((1, C), -float(C + MID - 1), dtype=np.float32)], axis=0
    ).astype(ml_dtypes.bfloat16)  # exact small ints

    iota = np.arange(C, dtype=np.float32).reshape(C, 1)
    biasc = b.astype(np.float32).reshape(C, 1)
    return wta, uzw, wz, iota, biasc


def kernel(reps, W, b, labels):
    from concourse.bass_utils import run_bass_kernel_spmd

    reps = np.asarray(reps, dtype=np.float32)
    W = np.asarray(W, dtype=np.float32)
    b = np.asarray(b, dtype=np.float32)
    labels_np = np.asarray(labels)

    if "nc" not in _CACHE:
        _CACHE["nc"] = _build_nc()
    nc = _CACHE["nc"]

    wta, uzw, wz, iota, biasc = _prepare_static(W, b)

    in_maps = []
    for core in range(NCORES):
        sh = slice(core * SHARD, (core + 1) * SHARD)
        lab = labels_np[sh].astype(np.float32)
        # device column order within a slab is (P, g, c) for batch row
        # (g*128 + P*32 + c); permute labels to match, per slab.
        pieces = []
        for base, rows in SLAB_DEFS:
            g = rows // 128
            pieces.append(
                lab[base : base + rows]
                .reshape(g, 4, 32)
                .transpose(1, 0, 2)
                .reshape(rows)
            )
        lab_perm = np.concatenate(pieces)
        lab_rep = np.broadcast_to(lab_perm, (C, SHARD)).copy()
        in_maps.append(
            {
                "reps": np.ascontiguousarray(reps[sh]),
                "labels_rep": lab_rep,
                "wta": wta,
                "uzw": uzw,
                "wz": wz,
                "iota": iota,
                "biasc": biasc,
            }
        )

    trace = bool(int(os.environ.get("CC_KERNEL_TRACE", "0")))
    res = run_bass_kernel_spmd(
        nc, in_maps, core_ids=list(range(NCORES)), trace=trace
    )
    if trace:
        _CACHE["last_results"] = res

    total = np.float64(0.0)
    for core in range(NCORES):
        total += np.float64(res.results[core]["partials"].sum(dtype=np.float64))
    loss = -(total / B)
    return np.float32(loss)
